# revision 13
# baseline (speedup 1.0000x reference)
"""MARL halftone REINFORCE loss on 8 Trainium2 NeuronCores.

Math (per batch image, all 512x512):
    e    = G*h - c            (G = 11x11 gaussian, SAME zero pad)
    corr = G*e
    reward = 2*delta*corr + delta^2*K2,  delta = 1-2h in {-1,+1} so delta^2 = 1
    lp   = log(p+eps) if h else log(1-p+eps) = ln|h+p-1| (+O(1e-6))
    loss = -sum_b sum_px (reward*lp) / B

Conv as banded matrix A (A[i,j] = gn[j-i+5], SAME-pad truncation at edges):
    G*x = A x A.   corr = A(AhA - c)A = B h B - A c A,  B = A@A (matrix product,
    edge-exact).  On the PE, op2(X; M) := X^T M, and op2(op2(X; M); M) = M X M
    with no transposes (M symmetric).  So the h-chain and c-chain run as two
    independent 2-pass pipelines.  Matmuls run in float32r (fp22) at full rate
    with 256-wide band windows.

Final reduction:
    sum(reward*lp)/(-8) = 0.5<T2, gt> - 0.5<S2, gt> - (K2/8)*sum(lp)
    where T2 = BhB, S2 = AcA, gt = (h-0.5)*lp = -delta*lp/2.
    <.,.> accumulated per-partition by fused scalar_tensor_tensor accum_out,
    and sum(lp) by the Ln activation's accum_out.  [128, 12] partials are
    DMA'd out per core and summed on the host.

Data parallel: core b handles image b.
"""

import numpy as np

B, HH, WW = 8, 512, 512
KSIZE = 11
SIGMA = 2.0
NCORES = 8
NBLK = 4  # 512 / 128
WIN = (0, 118, 246, 256)  # psum col window start per k-block, width 256
# rhs column offset inside the per-matrix band block (k0 / interior / k3 tiles)
BOFF = (0, 256, 256, 512)
ZCOL = 1536  # zero block columns [1536, 1792) in bands


def _gauss1d():
    ax = np.arange(KSIZE, dtype=np.float64) - (KSIZE - 1) / 2.0
    g = np.exp(-(ax ** 2) / (2.0 * SIGMA ** 2))
    return g / g.sum()


def _k2():
    gn = _gauss1d()
    k2d = np.outer(gn, gn)  # == outer(g,g)/sum(outer(g,g))
    return float(np.sum(k2d * k2d))


_np_cache = {}


def _bands_np():
    """[128, 1792] f32: A_k0|A_int|A_k3|B_k0|B_int|B_k3|zeros (256 cols each)."""
    if "bands" in _np_cache:
        return _np_cache["bands"]
    gn = _gauss1d()
    half = KSIZE // 2
    A = np.zeros((512, 512), dtype=np.float64)
    for o in range(-half, half + 1):
        idx = np.arange(max(0, -o), min(512, 512 - o))
        A[idx, idx + o] = gn[o + half]
    Bm = A @ A  # edge-exact double-conv matrix, band halfwidth 10

    def tiles(M):
        t = [M[128 * k: 128 * k + 128, WIN[k]: WIN[k] + 256] for k in range(4)]
        assert np.allclose(t[1], t[2], rtol=0, atol=1e-12), (
            "interior Toeplitz tiles must match"
        )
        return [t[0], t[1], t[3]]

    zero = np.zeros((128, 256), dtype=np.float64)
    bands = np.concatenate(tiles(A) + tiles(Bm) + [zero], axis=1).astype(np.float32)
    assert bands.shape == (128, 1792)
    _np_cache["bands"] = np.ascontiguousarray(bands)
    return _np_cache["bands"]


_module_cache = {}


def _build_module():
    if "nc" in _module_cache:
        return _module_cache["nc"]
    from contextlib import ExitStack

    import concourse.bass as bass  # noqa: F401
    import concourse.mybir as mybir
    import concourse.tile as tile
    from concourse import bacc

    f32 = mybir.dt.float32
    f32r = mybir.dt.float32r
    Alu = mybir.AluOpType
    Fn = mybir.ActivationFunctionType

    nc = bacc.Bacc("TRN2", target_bir_lowering=False, debug=True)

    h_d = nc.dram_tensor("h_in", [512, 512], f32r, kind="ExternalInput")
    c_d = nc.dram_tensor("c_in", [512, 512], f32r, kind="ExternalInput")
    p_d = nc.dram_tensor("p_in", [512, 512], f32r, kind="ExternalInput")
    bands_d = nc.dram_tensor("bands", [128, 1792], f32r, kind="ExternalInput")
    out_d = nc.dram_tensor("osum", [128, 12], f32, kind="ExternalOutput")

    with tile.TileContext(nc) as tc, ExitStack() as ctx:
        sb = ctx.enter_context(tc.tile_pool(name="sb", bufs=1))
        ps = ctx.enter_context(tc.tile_pool(name="ps", bufs=8, space="PSUM"))

        h_sb = sb.tile([128, 2048], f32r, name="h_sb")
        c_sb = sb.tile([128, 2048], f32r, name="c_sb")
        p_sb = sb.tile([128, 2048], f32r, name="p_sb")
        bands_sb = sb.tile([128, 1792], f32r, name="bands_sb")
        t1_sb = sb.tile([128, 2048], f32r, name="t1_sb")
        s1_sb = sb.tile([128, 2048], f32r, name="s1_sb")
        r_sb = sb.tile([128, 2048], f32, name="r_sb")
        ab_sb = sb.tile([128, 2048], f32, name="ab_sb")
        lp_sb = sb.tile([128, 2048], f32, name="lp_sb")
        g_sb = sb.tile([128, 2048], f32, name="g_sb")
        mt_sb = sb.tile([128, 2048], f32, name="mt_sb")
        ms_sb = sb.tile([128, 2048], f32, name="ms_sb")
        sums = sb.tile([128, 12], f32, name="sums")
        warm = sb.tile([1, 16], f32, name="warm")
        warm2 = sb.tile([1, 16], f32, name="warm2")
        neg_one = sb.tile([128, 1], f32, name="neg_one")

        # Preload the Ln activation table set during the DMA phase.
        nc.gpsimd.memset(warm[:], 1.0)
        nc.scalar.activation(warm2[:], warm[:], Fn.Ln)
        nc.gpsimd.memset(neg_one[:], -1.0)

        # --- input DMAs (HWDGE; program order == queue order) -------------
        def dma_blk(dst, src, k):
            nc.sync.dma_start(
                out=dst[:, 512 * k: 512 * (k + 1)],
                in_=src[128 * k: 128 * (k + 1), :],
            )

        dma_blk(p_sb, p_d, 0)
        nc.sync.dma_start(out=bands_sb[:, 768:1792], in_=bands_d[:, 768:1792])
        for k in range(4):
            dma_blk(h_sb, h_d, k)
        dma_blk(p_sb, p_d, 1)
        nc.sync.dma_start(out=bands_sb[:, 0:768], in_=bands_d[:, 0:768])
        dma_blk(c_sb, c_d, 0)
        dma_blk(c_sb, c_d, 1)
        dma_blk(p_sb, p_d, 2)
        dma_blk(c_sb, c_d, 2)
        dma_blk(c_sb, c_d, 3)
        dma_blk(p_sb, p_d, 3)

        zero256 = bands_sb[:, ZCOL: ZCOL + 256]

        def conv_pass(src, mat_off, out_tiles):
            """out[ib] = src^T M banded: 4 kb-groups x 4 banks.

            Bank init: the kb=0 window MM carries start=True (clears the whole
            bank's has_written bits, covers cols [0,256)); a zero-rhs MM then
            fills cols [256,512) so every element is TensorE-written before
            later windows accumulate.  Exact on HW, and keeps CoreSim's
            per-bank pending-zero model uniform per instruction.
            """
            for kb in range(4):
                rhs = bands_sb[:, mat_off + BOFF[kb]: mat_off + BOFF[kb] + 256]
                for ib in range(4):
                    lhsT = src[:, 512 * kb + 128 * ib: 512 * kb + 128 * ib + 128]
                    nc.tensor.matmul(
                        out_tiles[ib][:, WIN[kb]: WIN[kb] + 256],
                        lhsT,
                        rhs,
                        start=(kb == 0),
                        stop=(kb == 3),
                    )
                    if kb == 0:
                        nc.tensor.matmul(
                            out_tiles[ib][:, 256:512],
                            lhsT,
                            zero256,
                            start=False,
                            stop=False,
                        )

        # --- T chain: T2 = B h B ------------------------------------------
        tT1 = [ps.tile([128, 512], f32, name=f"tT1_{i}", tag="bank") for i in range(4)]
        conv_pass(h_sb, 768, tT1)
        for ib in range(4):
            nc.scalar.copy(t1_sb[:, 512 * ib: 512 * (ib + 1)], tT1[ib][:])
        tT2 = [ps.tile([128, 512], f32, name=f"tT2_{i}", tag="bank") for i in range(4)]
        conv_pass(t1_sb, 768, tT2)

        # --- S chain: S2 = A c A ------------------------------------------
        tS1 = [ps.tile([128, 512], f32, name=f"tS1_{i}", tag="bank") for i in range(4)]
        conv_pass(c_sb, 0, tS1)
        for ib in range(4):
            nc.scalar.copy(s1_sb[:, 512 * ib: 512 * (ib + 1)], tS1[ib][:])
        tS2 = [ps.tile([128, 512], f32, name=f"tS2_{i}", tag="bank") for i in range(4)]
        conv_pass(s1_sb, 0, tS2)

        # --- lp chain ------------------------------------------------------
        f32 = mybir.dt.float32
        for ib in range(4):
            s = slice(512 * ib, 512 * (ib + 1))
            hv = h_sb[:, s].bitcast(f32)
            pv = p_sb[:, s].bitcast(f32)
            # r = h + p
            nc.gpsimd.tensor_tensor(r_sb[:, s], hv, pv, Alu.add)
            # a = (r - 1)^2   (in [1e-4, 1])
            nc.scalar.activation(ab_sb[:, s], r_sb[:, s], Fn.Square, bias=neg_one[:])
            # lp2 = ln(a) = 2*lp, accumulate per-partition sum(2*lp)
            nc.scalar.activation(
                lp_sb[:, s], ab_sb[:, s], Fn.Ln,
                accum_out=sums[:, 8 + ib: 9 + ib],
            )
            # gt2 = (h - 0.5) * lp2  ( = -delta*lp )
            nc.vector.scalar_tensor_tensor(
                g_sb[:, s], hv, 0.5, lp_sb[:, s], Alu.subtract, Alu.mult
            )

        # --- final products + accumulation --------------------------------
        for ib in range(4):
            s = slice(512 * ib, 512 * (ib + 1))
            nc.vector.scalar_tensor_tensor(
                mt_sb[:, s], tT2[ib][:], 0.25, g_sb[:, s], Alu.mult, Alu.mult,
                accum_out=sums[:, ib: ib + 1],
            )
            nc.vector.scalar_tensor_tensor(
                ms_sb[:, s], tS2[ib][:], -0.25, g_sb[:, s], Alu.mult, Alu.mult,
                accum_out=sums[:, 4 + ib: 5 + ib],
            )

        nc.sync.dma_start(out=out_d[:], in_=sums[:])

    nc.finalize()
    _module_cache["nc"] = nc
    return nc


def _in_maps(prob_map, c, h_sampled):
    bands = _bands_np()
    maps = []
    for b in range(B):
        maps.append(
            {
                "h_in": np.ascontiguousarray(h_sampled[b, 0], dtype=np.float32),
                "c_in": np.ascontiguousarray(c[b, 0], dtype=np.float32),
                "p_in": np.ascontiguousarray(prob_map[b, 0], dtype=np.float32),
                "bands": bands,
            }
        )
    return maps


def _reduce_host(results):
    k2 = _k2()
    total = 0.0
    for r in results:
        o = np.asarray(r["osum"], dtype=np.float64)
        total += o[:, 0:8].sum() - (k2 / 16.0) * o[:, 8:12].sum()
    return np.float32(total)


def kernel(prob_map, c, h_sampled, **kw_extra):
    from concourse.bass_utils import run_bass_kernel_spmd

    nc = _build_module()
    maps = _in_maps(prob_map, c, h_sampled)
    res = run_bass_kernel_spmd(nc, maps, core_ids=list(range(NCORES)))
    return _reduce_host(res.results)
